# revision 9
# baseline (speedup 1.0000x reference)
"""Trainium2 kernel for nn_Circuit_41936060678727.

The reference is a 10-qubit real-amplitude circuit (CNOT ladders + RY
rotations), measured with PauliZ on each wire.  Every gate is linear, so the
circuit collapses to one 784x1024 matrix W with orthonormal rows:

    out[b, p] = sum_z sign_p(z) y_z^2 / ||y||^2,   y = W^T x_b

fp8 trick: x is uniform[0,1), so ~87% of each sample's energy sits in the
all-ones direction u.  Split x = m*u + xt (xt = x - mean, m = u^T x exact on
host).  Then with q = W^T u (||q||=1, exact):

    num_p = m^2 * alpha_p + 2 m * (gamma_p . xt) + sum_z s_pz yt_z^2
    den   = m^2 + ||yt||^2                     (u _|_ xt  =>  q _|_ yt exactly)

where alpha_p = sum_z s_pz q_z^2 (host, exact), gamma_p = W (s_p * q) (host,
quantized hi/lo fp8), yt = W^T xt.  Removing the mean shrinks the operand
~2.8x, which makes e4m3 DoubleRow matmuls (2x PE rate, K=256/pass) accurate
enough: simulated rel err 9.4e-3 vs the 2e-2 gate.

Device per core (batch 2048, groups of 512):
    mm1: yt = Wq^T x_hi          fp8 DR, K=1024(pad), 8 z-chunks, 128 MMs
    sq  = yt^2                   ACT/DVE split, fp8 out (|yt|<=11.6 -> sq<=135)
    mm2: po = [S|1]^T sq         fp8 DR, po strips share one PSUM bank whose
                                 has_written bits are pre-set by a zero matmul
    t2: gamma^T xt hi/lo trio    fp8 DR, M=32/16, transient PSUM tiles
Host: assemble num/den, divide.
"""

import numpy as np
import ml_dtypes

N_QUBITS = 10
DIM = 1 << N_QUBITS          # 1024
N_OUT = 10
D_IN = 784
B_TOTAL = 16384
N_CORES = 8
B_CORE = B_TOTAL // N_CORES  # 2048
GROUP = 512
N_GROUPS = B_CORE // GROUP   # 4
KSUB = 8                     # 1024 contraction rows = 8 subtiles of 128
NZ = 8                       # 1024 output states = 8 chunks of 128

S_X = 2.0                    # x_hi = e4m3(S_X * xt)
S_W = 4.0                    # Wq   = e4m3(S_W * W)   (S_X*S_W=8 keeps sq<240)
S_G = 16.0                   # g_hi = e4m3(S_G * gamma)
LO = 16.0                    # lo-residual upscale

F8 = ml_dtypes.float8_e4m3   # TRN FP8_EXP4 semantics (max 240, inf at 256)


# ----------------------------------------------------------------------------
# Host-side precompute
# ----------------------------------------------------------------------------

def _apply_ry(S, theta, q):
    B = S.shape[0]
    left, right = 1 << q, 1 << (N_QUBITS - q - 1)
    s = S.reshape(B, left, 2, right)
    c, sn = np.cos(theta / 2), np.sin(theta / 2)
    s0 = c * s[:, :, 0] - sn * s[:, :, 1]
    s1 = sn * s[:, :, 0] + c * s[:, :, 1]
    return np.stack([s0, s1], axis=2).reshape(B, DIM)


def _apply_cnot(S, q):
    B = S.shape[0]
    left, right = 1 << q, 1 << (N_QUBITS - q - 2)
    s = S.reshape(B, left, 2, 2, right)
    s = np.concatenate([s[:, :, :1], np.flip(s[:, :, 1:], axis=3)], axis=2)
    return s.reshape(B, DIM)


def _build_W(params):
    """Circuit applied to basis rows e_0..e_783 -> W[784, 1024], fp64."""
    w = np.pi * np.tanh(params.astype(np.float64))
    S = np.zeros((D_IN, DIM), dtype=np.float64)
    S[np.arange(D_IN), np.arange(D_IN)] = 1.0
    for l in range(params.shape[0]):
        for start in (0, 1):
            for i in range(start, N_QUBITS - 1, 2):
                S = _apply_cnot(S, i)
        for i in range(N_QUBITS):
            S = _apply_ry(S, w[l, i], i)
    return S


def _sign_matrix():
    z = np.arange(DIM)
    S = np.zeros((DIM, N_OUT), dtype=np.float64)
    for p in range(N_OUT):
        S[:, p] = 1.0 - 2.0 * ((z >> (N_QUBITS - 1 - p)) & 1)
    return S


def _e4(a):
    return np.asarray(a, np.float32).astype(F8)


def _pack_k_major(a):
    """[1024, cols] -> [128, 8, cols] with [p, s, :] = a[128 s + p, :]."""
    cols = a.shape[1]
    return np.ascontiguousarray(
        a.reshape(KSUB, 128, cols).transpose(1, 0, 2)
    )


# ----------------------------------------------------------------------------
# Bass program (identical SPMD program on all 8 cores)
# ----------------------------------------------------------------------------

_NC_CACHE = {}
TRACE = False
LAST_RESULTS = None


def _build_bass():
    from contextlib import ExitStack

    import concourse.tile as tile
    from concourse import bacc, mybir

    f32 = mybir.dt.float32
    f8 = mybir.dt.float8e4
    bf16 = mybir.dt.bfloat16
    DR = mybir.MatmulPerfMode.DoubleRow

    nc = bacc.Bacc(
        "TRN2", target_bir_lowering=False, debug=False, num_devices=N_CORES
    )
    zs_d = nc.declare_dram_parameter("zs", [128, KSUB, 16], f8, isOutput=False)
    zsb_d = nc.declare_dram_parameter("zsb", [128, KSUB, 16], bf16, isOutput=False)
    ghl_d = nc.declare_dram_parameter("ghl", [128, KSUB, 32], f8, isOutput=False)
    xh_d = nc.declare_dram_parameter("xh", [128, KSUB, B_CORE], f8, isOutput=False)
    wq_d = nc.declare_dram_parameter("wq", [NZ * 128, KSUB, 128], f8, isOutput=False)
    xl_d = nc.declare_dram_parameter("xl", [128, KSUB, B_CORE], f8, isOutput=False)
    out_t2_d = nc.declare_dram_parameter("out_t2", [80, B_CORE], f32, isOutput=True)

    def mm(out, lhsT, rhs, start, stop, tile_position=None):
        nc.tensor.matmul(
            out, lhsT=lhsT, rhs=rhs, start=start, stop=stop,
            perf_mode=DR, skip_group_check=True, tile_position=tile_position,
        )

    with ExitStack() as ctx:
        tc = ctx.enter_context(tile.TileContext(nc))
        gz = ctx.enter_context(tc.tile_pool(name="gz", bufs=1))
        xpool = ctx.enter_context(tc.tile_pool(name="x", bufs=1))
        xlpool = ctx.enter_context(tc.tile_pool(name="xl", bufs=1))
        wpool = ctx.enter_context(tc.tile_pool(name="w", bufs=1))
        sqpool = ctx.enter_context(tc.tile_pool(name="sq", bufs=5))
        ybfpool = ctx.enter_context(tc.tile_pool(name="ybf", bufs=3))
        osb = ctx.enter_context(tc.tile_pool(name="osb", bufs=1))
        pypool = ctx.enter_context(tc.tile_pool(name="py", bufs=8, space="PSUM"))

        # --- input DMAs on the sync ring, in consumption order ---
        zs = gz.tile([128, KSUB, 16], f8, tag="zs")
        nc.sync.dma_start(zs[:], zs_d[:, :, :])
        zsb = gz.tile([128, KSUB, 16], bf16, tag="zsb")
        nc.sync.dma_start(zsb[:], zsb_d[:, :, :])
        ghl = gz.tile([128, KSUB, 32], f8, tag="ghl")
        nc.sync.dma_start(ghl[:], ghl_d[:, :, :])
        x_sb = []
        for g in range(N_GROUPS):
            t = xpool.tile([128, KSUB, GROUP], f8, tag=f"xh{g}", name=f"xh{g}")
            nc.sync.dma_start(t[:], xh_d[:, :, g * GROUP:(g + 1) * GROUP])
            x_sb.append(t)
        w_sb = []
        for z in range(NZ):
            t = wpool.tile([128, KSUB, 128], f8, tag=f"w{z}", name=f"w{z}")
            nc.sync.dma_start(t[:], wq_d[z * 128:(z + 1) * 128, :, :])
            w_sb.append(t)
        xl_sb = []
        for g in range(N_GROUPS):
            t = xlpool.tile([128, KSUB, GROUP], f8, tag=f"xl{g}", name=f"xl{g}")
            nc.sync.dma_start(t[:], xl_d[:, :, g * GROUP:(g + 1) * GROUP])
            xl_sb.append(t)

        out_t2_sb = osb.tile([80, B_CORE], f32, tag="ot2")

        # --- prefix: PE warmup + t2a while x groups land ---
        warm = pypool.tile([128, GROUP], f32, tag="py", name="warm")

        def warmups(n):
            for _ in range(n):
                nc.tensor.matmul(
                    warm[0:32, 0:32], lhsT=zs[:, 0:2, :], rhs=zs[:, 2:4, :],
                    start=True, stop=True, skip_group_check=True,
                )

        warmups(12)
        for g in range(N_GROUPS):
            t2a = pypool.tile([128, GROUP], f32, tag="py", name=f"t2a{g}")
            for c in range(4):
                mm(t2a[0:32, :], ghl[:, 2 * c:2 * c + 2, :],
                   x_sb[g][:, 2 * c:2 * c + 2, :], start=(c == 0), stop=(c == 3))
            eng = nc.scalar if g < 2 else nc.vector
            if g < 2:
                eng.copy(out_t2_sb[0:32, g * GROUP:(g + 1) * GROUP], t2a[0:32, :])
            else:
                eng.tensor_copy(out_t2_sb[0:32, g * GROUP:(g + 1) * GROUP], t2a[0:32, :])
            nc.scalar.dma_start(
                out_t2_d[0:32, g * GROUP:(g + 1) * GROUP],
                out_t2_sb[0:32, g * GROUP:(g + 1) * GROUP],
            )
            warmups(12)

        # --- main loop over the 8 output-state chunks ---
        # square engine split: ACT does fp8 squares straight from PSUM; DVE
        # cannot read PSUM twice (one port), so its share is copy->bf16 then
        # a 2x-mode SBUF tensor_tensor square, and mm2 takes those chunks as
        # plain bf16 matmuls.  DVE-pairs: pair 3 for g<2, pairs 2-3 for g>=2.
        def dve_pair(pair, g):
            return pair == 3 if g < 2 else pair >= 2

        sq_tiles = {}
        for z in range(NZ):
            pair = z // 2
            pys = [
                pypool.tile([128, GROUP], f32, tag="py", name=f"py{z}_{g}")
                for g in range(N_GROUPS)
            ]
            for c in range(4):
                for g in range(N_GROUPS):
                    mm(pys[g][:], w_sb[z][:, 2 * c:2 * c + 2, :],
                       x_sb[g][:, 2 * c:2 * c + 2, :],
                       start=(c == 0), stop=(c == 3))
            for g in range(N_GROUPS):
                dve = dve_pair(pair, g)
                if z % 2 == 0:
                    sq_tiles[g] = sqpool.tile(
                        [128, 2, GROUP], bf16 if dve else f8,
                        tag="sqb" if dve else "sq", name=f"sq{pair}_{g}",
                        bufs=6 if dve else 10,
                    )
                sq_tiles[(pair, g)] = sq_tiles[g]
                dst = sq_tiles[g][:, z % 2, :]
                if dve:
                    ybf = ybfpool.tile([128, GROUP], bf16, tag="ybf",
                                       name=f"ybf{z}_{g}")
                    nc.vector.tensor_copy(ybf[:], pys[g][:])
                    nc.vector.tensor_tensor(
                        dst, ybf[:], ybf[:], mybir.AluOpType.mult
                    )
                else:
                    nc.scalar.square(dst, pys[g][:])
            # t2b once x_lo has landed (after z=4/z=5)
            if z in (4, 5):
                for g in (0, 1) if z == 4 else (2, 3):
                    t2b = pypool.tile([128, GROUP], f32, tag="py", name=f"t2b{g}")
                    for c in range(4):
                        mm(t2b[0:16, :], ghl[:, 2 * c:2 * c + 2, 0:16],
                           xl_sb[g][:, 2 * c:2 * c + 2, :],
                           start=(c == 0), stop=(c == 3))
                    if g < 2:
                        nc.scalar.copy(
                            out_t2_sb[32:48, g * GROUP:(g + 1) * GROUP], t2b[0:16, :]
                        )
                    else:
                        nc.vector.tensor_copy(
                            out_t2_sb[32:48, g * GROUP:(g + 1) * GROUP], t2b[0:16, :]
                        )
                    nc.scalar.dma_start(
                        out_t2_d[32:48, g * GROUP:(g + 1) * GROUP],
                        out_t2_sb[32:48, g * GROUP:(g + 1) * GROUP],
                    )

        # --- tail: per-group sign-sum matmuls (DR needs dst partition 0) ---
        for g in range(N_GROUPS):
            po = pypool.tile([128, GROUP], f32, tag="py", name=f"po{g}")
            first = True
            for pair in range(4):
                sqt = sq_tiles[(pair, g)]
                if dve_pair(pair, g):
                    for i in range(2):
                        nc.tensor.matmul(
                            po[0:16, :], lhsT=zsb[:, 2 * pair + i, :],
                            rhs=sqt[:, i, :], start=first,
                            stop=(pair == 3 and i == 1),
                            skip_group_check=True,
                        )
                        first = False
                else:
                    mm(po[0:16, :], zs[:, 2 * pair:2 * pair + 2, :],
                       sqt[:], start=first, stop=(pair == 3))
                    first = False
            if g < 2:
                nc.scalar.copy(
                    out_t2_sb[64:80, g * GROUP:(g + 1) * GROUP], po[0:16, :]
                )
            else:
                nc.vector.tensor_copy(
                    out_t2_sb[64:80, g * GROUP:(g + 1) * GROUP], po[0:16, :]
                )
            nc.scalar.dma_start(
                out_t2_d[64:80, g * GROUP:(g + 1) * GROUP],
                out_t2_sb[64:80, g * GROUP:(g + 1) * GROUP],
            )

    nc.finalize()
    return nc


def _get_nc():
    if "nc" not in _NC_CACHE:
        _NC_CACHE["nc"] = _build_bass()
    return _NC_CACHE["nc"]


# ----------------------------------------------------------------------------
# Entry point
# ----------------------------------------------------------------------------

def kernel(input, params):
    global LAST_RESULTS
    from concourse.bass_utils import run_bass_kernel_spmd

    x = np.asarray(input, dtype=np.float64)
    p = np.asarray(params, dtype=np.float32)
    B = x.shape[0]
    assert B == B_TOTAL and x.shape[1] == D_IN

    W = _build_W(p)                       # [784, 1024] fp64
    S = _sign_matrix()                    # [1024, 10]
    u = np.ones(D_IN) / np.sqrt(D_IN)
    q = W.T @ u                           # [1024]
    alpha = (S * (q**2)[:, None]).sum(axis=0)          # [10]
    gamma = W @ (S * q[:, None])                       # [784, 10]

    # per-sample DC split (host, exact fp64)
    m = x @ u                                           # [B]
    xt = (x - m[:, None] * u[None, :]).astype(np.float32)

    # quantize + pack weights
    Wp = np.zeros((DIM, DIM), dtype=np.float32)
    Wp[:D_IN] = S_W * W.astype(np.float32)
    W8 = _e4(Wp)                                        # [1024, 1024]
    wq_host = np.ascontiguousarray(
        W8.reshape(KSUB, 128, NZ, 128).transpose(2, 1, 0, 3).reshape(NZ * 128, KSUB, 128)
    )

    G = np.zeros((DIM, 32), dtype=np.float32)
    G[:D_IN, 0:N_OUT] = S_G * gamma
    g_hi = _e4(G)
    g_res = np.zeros((DIM, 32), dtype=np.float32)
    g_res[:, 16:16 + N_OUT] = LO * (
        G[:, 0:N_OUT] - g_hi[:, 0:N_OUT].astype(np.float32)
    )
    ghl_host = _pack_k_major(
        (g_hi.astype(np.float32) + g_res).astype(np.float32)
    )
    ghl_host = _e4(ghl_host)

    Z = np.zeros((DIM, 16), dtype=np.float32)
    Z[:, :N_OUT] = S
    Z[:, N_OUT] = 1.0
    zs_host = _e4(_pack_k_major(Z))
    zsb_host = _pack_k_major(Z).astype(ml_dtypes.bfloat16)

    # quantize + pack x (hi/lo)
    xtT = np.zeros((DIM, B), dtype=np.float32)
    xtT[:D_IN] = S_X * xt.T
    x_hi = _e4(xtT)                                     # [1024, B]
    x_lo = _e4(LO * (xtT - x_hi.astype(np.float32)))
    xh_all = _pack_k_major(x_hi)                        # [128, 8, B]
    xl_all = _pack_k_major(x_lo)

    nc = _get_nc()
    in_maps = []
    for c in range(N_CORES):
        sl = slice(c * B_CORE, (c + 1) * B_CORE)
        in_maps.append({
            "zs": zs_host,
            "zsb": zsb_host,
            "ghl": ghl_host,
            "xh": np.ascontiguousarray(xh_all[:, :, sl]),
            "wq": wq_host,
            "xl": np.ascontiguousarray(xl_all[:, :, sl]),
        })

    res = run_bass_kernel_spmd(nc, in_maps, list(range(N_CORES)), trace=TRACE)
    LAST_RESULTS = res

    SC2 = (S_X * S_W) ** 2
    outs = np.empty((B, N_OUT), dtype=np.float64)
    for c in range(N_CORES):
        t2r = res.results[c]["out_t2"].astype(np.float64)   # [64, 2048]
        t3 = t2r[64:64 + N_OUT].T / SC2
        n3 = t2r[64 + N_OUT] / SC2
        t2 = (
            t2r[0:N_OUT].T / (S_X * S_G)
            + (t2r[16:16 + N_OUT].T + t2r[32:32 + N_OUT].T) / (LO * S_X * S_G)
        )
        mc = m[c * B_CORE:(c + 1) * B_CORE]
        num = mc[:, None] ** 2 * alpha[None, :] + 2 * mc[:, None] * t2 + t3
        den = mc**2 + n3
        outs[c * B_CORE:(c + 1) * B_CORE] = num / den[:, None]

    return np.ascontiguousarray(outs.astype(np.float32))
